# revision 14
# baseline (speedup 1.0000x reference)
"""Trainium2 Bass kernel for LucidrainsLFQ (lookup-free quantization).

Reference computation (per token, C=512 channels, D=14 codebook bits):
  y      = W_in @ z_e + b_in                      (project in, 14 dims)
  quant  = sign(y)                                (+-1)
  z_q    = W_out @ quant + b_out                  (project out)
  The softmax over the 2^14 implicit codebook factorizes: the codebook is
  all sign patterns of {-1,+1}^14, so softmax(200 * y . c_j) over j is a
  product of 14 independent Bernoullis.  Split 14 bits = 7 + 7:
     prob[jA, jB] = A[jA] * B[jB]   with A, B 128-way softmaxes
  avg_prob = mean_n A_n outer B_n = (A^T B)/N  -> one 128x128 matmul.
  per-sample entropy = sum_d H_b(sigmoid(400 y_d))  (Bernoulli entropies)
  commit = mean((|y| - 1)^2)
  usage  = distinct hard codes / 16384

Sharding: data-parallel over the 8192 tokens; each of 8 cores handles one
(batch b, 1024-token slice).  Tiny weights/codebook are replicated.  Each
core returns its z_q slice plus partial sums (avg-prob outer product,
entropy/commit sums, hard indices); the host combines the partials into
the two scalars.

Kernel structure: two 512-token chunks pipelined (DMA in / project /
quantize / project out / DMA out), with the A/B softmax + avg-prob
accumulation and the entropy/commit reductions running per chunk on
whichever engines are free.  b_out is folded into the out-projection as a
15th contraction row whose activation is constant 1.
"""

import numpy as np

B, C, T = 2, 512, 4096
D = 14
NCORES = 8
TCORE = (B * T) // NCORES  # 1024
CHUNK = 512
NCHUNK = TCORE // CHUNK
INV_TEMP = 100.0
ENTROPY_W = 0.1
COMMIT_W = 0.25
DIVERSITY_GAMMA = 1.0
EPS = 1e-20

_CACHE = {}


def _build_module():
    import concourse.bacc as bacc
    import concourse.bass as bass
    import concourse.mybir as mybir
    import concourse.tile as tile

    f32 = mybir.dt.float32
    bf16 = mybir.dt.bfloat16
    Act = mybir.ActivationFunctionType
    Alu = mybir.AluOpType
    X = mybir.AxisListType.X

    nc = bacc.Bacc("TRN2", target_bir_lowering=False, debug=False,
                   num_devices=NCORES)

    ze_d = nc.dram_tensor("z_part", (C, TCORE), f32, kind="ExternalInput")
    win_d = nc.dram_tensor("w_in_c", (128, 4 * D), f32, kind="ExternalInput")
    wout_d = nc.dram_tensor("w_outT", (D + 1, C), f32, kind="ExternalInput")
    cst_d = nc.dram_tensor("consts", (D, 259), f32, kind="ExternalInput")
    ones_d = nc.dram_tensor("ones_row", (1, TCORE), f32, kind="ExternalInput")

    zq_d = nc.dram_tensor("zq_part", (C, TCORE), f32, kind="ExternalOutput")
    avg_d = nc.dram_tensor("avg_part", (128, 128), f32, kind="ExternalOutput")
    stats_d = nc.dram_tensor("stats", (D, 2 * NCHUNK), f32,
                             kind="ExternalOutput")
    idx_d = nc.dram_tensor("idx", (1, TCORE), f32, kind="ExternalOutput")

    ze_r = ze_d.ap().rearrange("(c p) (j t) -> p c j t", p=128, t=CHUNK)
    zq_r = zq_d.ap().rearrange("(c p) (j t) -> p c j t", p=128, t=CHUNK)

    with tile.TileContext(nc) as tc:
        with (
            tc.tile_pool(name="const", bufs=1) as cpool,
            tc.tile_pool(name="data", bufs=1) as dpool,
            tc.tile_pool(name="chunkio", bufs=2) as iopool,
            tc.tile_pool(name="work", bufs=2) as wpool,
            tc.tile_pool(name="ab", bufs=3) as abpool,
            tc.tile_pool(name="ps", bufs=2, space=bass.MemorySpace.PSUM) as pspool,
            tc.tile_pool(name="ps1", bufs=1, space=bass.MemorySpace.PSUM) as ps1pool,
            tc.tile_pool(name="psavg", bufs=1, space=bass.MemorySpace.PSUM) as avgpool,
        ):
            win_t = cpool.tile([128, 4 * D], f32)
            nc.sync.dma_start(win_t[:], win_d.ap())
            wout_t = cpool.tile([D + 1, C], f32)
            nc.sync.dma_start(wout_t[:], wout_d.ap())
            cst_t = cpool.tile([D, 259], f32)
            nc.sync.dma_start(cst_t[:], cst_d.ap())

            ones_bf = cpool.tile([D, 1], bf16)
            nc.gpsimd.memset(ones_bf[:], 1.0)

            y_t = dpool.tile([D, TCORE], f32)
            q_t = dpool.tile([D + 1, TCORE], f32)
            nc.sync.dma_start(q_t[D:D + 1, :], ones_d.ap())
            idx_t = dpool.tile([1, TCORE], f32)
            stats_t = dpool.tile([D, 2 * NCHUNK], f32)
            avg_ps = avgpool.tile([128, 128], f32)

            for j in range(NCHUNK):
                cs = slice(CHUNK * j, CHUNK * (j + 1))

                ze_t = iopool.tile([128, 4, CHUNK], f32, tag="ze")
                nc.sync.dma_start(ze_t[:], ze_r[:, :, j])

                # ---- project in: y[d, t] = sum_c W_in[d, c] z_e[c, t] ----
                yp = pspool.tile([D, CHUNK], f32, tag="ypsum")
                for c in range(4):
                    nc.tensor.matmul(yp[:], win_t[:, D * c:D * (c + 1)],
                                     ze_t[:, c, :],
                                     start=(c == 0), stop=(c == 3))
                nc.scalar.activation(y_t[:, cs], yp[:], Act.Identity,
                                     bias=cst_t[:, 258:259])

                # ---- quantize; weighted bits for the hard index ----
                bits_t = wpool.tile([D, CHUNK], f32, tag="bits")
                nc.vector.tensor_scalar(bits_t[:], y_t[:, cs], 0.0, None,
                                        op0=Alu.is_gt)
                nc.vector.tensor_scalar(q_t[0:D, cs], bits_t[:], 2.0, -1.0,
                                        op0=Alu.mult, op1=Alu.add)
                wb_t = wpool.tile([D, CHUNK], bf16, tag="wb")
                nc.vector.tensor_scalar_mul(wb_t[:], bits_t[:],
                                            cst_t[:, 256:257])

                # ---- project out (b_out folded in as 15th row) ----
                zq_t = iopool.tile([128, 4, CHUNK], f32, tag="zq")
                for c in range(4):
                    zp = pspool.tile([128, CHUNK], f32, tag="zqpsum")
                    nc.tensor.matmul(zp[:], wout_t[:, 128 * c:128 * (c + 1)],
                                     q_t[:, cs])
                    if c % 2 == 0:
                        nc.scalar.copy(zq_t[:, c, :], zp[:])
                    else:
                        nc.vector.tensor_copy(zq_t[:, c, :], zp[:])
                nc.scalar.dma_start(zq_r[:, :, j], zq_t[:])

                # ---- hard index: sum_d bit_d * 2^(13-d) ----
                ip = ps1pool.tile([1, CHUNK], f32, tag="idxpsum")
                nc.tensor.matmul(ip[:], ones_bf[:], wb_t[:])
                nc.vector.tensor_copy(idx_t[:, cs], ip[:])

                # ---- A/B half-softmaxes; avg_prob outer-product partial ----
                for tt in range(CHUNK // 128):
                    gt = j * (CHUNK // 128) + tt
                    Lp = pspool.tile([128, 2, 128], f32, tag="Lpsum")
                    nc.tensor.matmul(
                        Lp[:].rearrange("p a b -> p (a b)"),
                        y_t[:, 128 * gt:128 * (gt + 1)],
                        cst_t[:, 0:256],
                    )
                    m2 = abpool.tile([128, 2], f32, tag="m2")
                    nc.vector.tensor_reduce(m2[:], Lp[:], axis=X,
                                            op=Alu.max, negate=True)
                    E_t = abpool.tile([128, 2, 128], bf16, tag="E")
                    s2 = abpool.tile([128, 2], f32, tag="s2")
                    for h in range(2):
                        nc.scalar.activation(E_t[:, h, :], Lp[:, h, :], Act.Exp,
                                             bias=m2[:, h:h + 1],
                                             accum_out=s2[:, h:h + 1])
                    sprod = abpool.tile([128, 1], f32, tag="sprod")
                    nc.vector.tensor_tensor(sprod[:], s2[:, 0:1], s2[:, 1:2],
                                            op=Alu.mult)
                    rr = abpool.tile([128, 1], f32, tag="rr")
                    nc.vector.reciprocal(rr[:], sprod[:])
                    Ap_t = abpool.tile([128, 128], bf16, tag="Ap")
                    nc.vector.tensor_scalar_mul(Ap_t[:], E_t[:, 0, :], rr[:])
                    nc.tensor.matmul(avg_ps[:], Ap_t[:], E_t[:, 1, :],
                                     start=(gt == 0),
                                     stop=(gt == TCORE // 128 - 1))

                # ---- Bernoulli entropy + commitment partial sums ----
                # a = |y|, p = sigmoid(400 a):  H_b = -ln(p) + 400 a (1 - p)
                a_t = wpool.tile([D, CHUNK], f32, tag="a")
                nc.scalar.activation(a_t[:], y_t[:, cs], Act.Abs)
                p_t = wpool.tile([D, CHUNK], f32, tag="p")
                nc.scalar.activation(p_t[:], a_t[:], Act.Sigmoid, scale=400.0)
                lp_t = wpool.tile([D, CHUNK], f32, tag="lp")
                nc.scalar.activation(lp_t[:], p_t[:], Act.Ln)
                t_t = wpool.tile([D, CHUNK], f32, tag="t")
                nc.vector.tensor_tensor(t_t[:], a_t[:], p_t[:], op=Alu.mult)
                d_t = wpool.tile([D, CHUNK], f32, tag="d")
                nc.vector.tensor_tensor(d_t[:], a_t[:], t_t[:], op=Alu.subtract)
                scr_t = wpool.tile([D, CHUNK], f32, tag="scr")
                nc.vector.scalar_tensor_tensor(scr_t[:], d_t[:], 400.0, lp_t[:],
                                               op0=Alu.mult, op1=Alu.subtract)
                nc.vector.tensor_reduce(stats_t[:, j:j + 1], scr_t[:], axis=X,
                                        op=Alu.add)
                c1_t = wpool.tile([D, CHUNK], f32, tag="c1")
                nc.vector.tensor_scalar_add(c1_t[:], a_t[:], -1.0)
                sq_t = wpool.tile([D, CHUNK], f32, tag="sq")
                nc.scalar.activation(sq_t[:], c1_t[:], Act.Square,
                                     accum_out=stats_t[:, NCHUNK + j:NCHUNK + j + 1])

            avg_t = wpool.tile([128, 128], f32)
            nc.vector.tensor_copy(avg_t[:], avg_ps[:])
            nc.scalar.dma_start(avg_d.ap(), avg_t[:])
            nc.scalar.dma_start(stats_d.ap(), stats_t[:])
            nc.scalar.dma_start(idx_d.ap(), idx_t[:])

    nc.compile()
    return nc


def _host_consts(b_in):
    # consts layout (14, 259):
    #   [:, 0:256]  block-diagonal scaled half-codebooks:
    #       rows 0:7  cols   0:128 = 200 * C7T ; rows 7:14 cols 128:256 = 200 * C7T
    #   [:, 256] 2^(13-d)   [:, 257] unused   [:, 258] b_in
    c7t = ((((np.arange(128)[None, :] >> np.arange(6, -1, -1)[:, None]) & 1)
            * 2.0 - 1.0)).astype(np.float32)  # (7, 128)
    consts = np.zeros((D, 259), np.float32)
    consts[0:7, 0:128] = 2.0 * INV_TEMP * c7t
    consts[7:14, 128:256] = 2.0 * INV_TEMP * c7t
    consts[:, 256] = (1 << np.arange(D - 1, -1, -1)).astype(np.float32)
    consts[:, 258] = b_in
    return consts


def _host_inputs(z_e, W_in, b_in, W_out, b_out):
    w_in_c = np.ascontiguousarray(
        W_in.T.reshape(4, 128, D).transpose(1, 0, 2).reshape(128, 4 * D))
    w_outT = np.empty((D + 1, C), np.float32)
    w_outT[0:D] = W_out.T
    w_outT[D] = b_out
    consts = _host_consts(b_in)
    in_maps = []
    for k in range(NCORES):
        b, s = divmod(k, NCORES // B)
        zp = np.ascontiguousarray(z_e[b, :, s * TCORE:(s + 1) * TCORE])
        in_maps.append({"z_part": zp, "w_in_c": w_in_c, "w_outT": w_outT,
                        "consts": consts,
                        "ones_row": np.ones((1, TCORE), np.float32)})
    return in_maps


def kernel(z_e, W_in, b_in, W_out, b_out):
    from concourse import bass_utils

    z_e = np.ascontiguousarray(np.asarray(z_e, np.float32))
    W_in = np.asarray(W_in, np.float32)
    b_in = np.asarray(b_in, np.float32)
    W_out = np.asarray(W_out, np.float32)
    b_out = np.asarray(b_out, np.float32)

    if "nc" not in _CACHE:
        _CACHE["nc"] = _build_module()
    nc = _CACHE["nc"]

    in_maps = _host_inputs(z_e, W_in, b_in, W_out, b_out)
    res = bass_utils.run_bass_kernel_spmd(nc, in_maps, core_ids=list(range(NCORES)))
    results = res.results

    z_q = np.empty((B, C, T), np.float32)
    avg_sum = np.zeros((128, 128), np.float64)
    ent_sum = 0.0
    commit_sum = 0.0
    idx_all = []
    for k in range(NCORES):
        b, s = divmod(k, NCORES // B)
        r = results[k]
        z_q[b, :, s * TCORE:(s + 1) * TCORE] = r["zq_part"]
        avg_sum += r["avg_part"].astype(np.float64)
        ent_sum += float(r["stats"][:, 0:NCHUNK].sum(dtype=np.float64))
        commit_sum += float(r["stats"][:, NCHUNK:].sum(dtype=np.float64))
        idx_all.append(r["idx"].ravel())

    n = B * T
    avg_prob = avg_sum / n
    cb_ent = float(-np.sum(avg_prob * np.log(np.clip(avg_prob, EPS, None))))
    ps_ent = ent_sum / n
    commit = commit_sum / (n * D)
    aux = (ps_ent - DIVERSITY_GAMMA * cb_ent) * ENTROPY_W + commit * COMMIT_W

    idx = np.concatenate(idx_all).astype(np.int64)
    usage = len(np.unique(idx)) / 16384.0

    return (z_q, np.float32(aux), np.float32(usage))


# revision 17
# speedup vs baseline: 1.1529x; 1.1529x over previous
"""Trainium2 Bass kernel for LucidrainsLFQ (lookup-free quantization).

Reference computation (per token, C=512 channels, D=14 codebook bits):
  y      = W_in @ z_e + b_in                      (project in, 14 dims)
  quant  = sign(y)                                (+-1)
  z_q    = W_out @ quant + b_out                  (project out)
  The softmax over the 2^14 implicit codebook factorizes: the codebook is
  all sign patterns of {-1,+1}^14, so softmax(200 * y . c_j) over j is a
  product of 14 independent Bernoullis.  Split 14 bits = 7 + 7:
     prob[jA, jB] = A[jA] * B[jB]   with A, B 128-way softmaxes
  avg_prob = mean_n A_n outer B_n = (A^T B)/N  -> one 128x128 matmul.
  per-sample entropy = sum_d H_b(sigmoid(400 y_d))  (Bernoulli entropies)
  commit = mean((|y| - 1)^2)
  usage  = distinct hard codes / 16384

Sharding: data-parallel over the 8192 tokens; each of 8 cores handles one
(batch b, 1024-token slice).  Tiny weights/codebook are replicated.  Each
core returns its z_q slice plus partial sums (avg-prob outer product,
entropy/commit sums, hard indices); the host combines the partials.

Performance structure:
  * two 512-token chunks pipelined; z_e arrives in 256 KB per-C-chunk DMAs
    so the first in-projection matmul starts as early as possible; consts
    ride the GpSimd SWDGE ring so they don't serialize behind z_e on SP.
  * dummy matmuls warm the PE clock (HAM) during the initial DMA window.
  * out-projection runs in bf16 (quant is exactly representable; W_out
    rounding contributes ~4e-4 relative on z_q); b_out is folded in as a
    15th contraction row whose activation is constant 1.
  * entropy/commit run post-loop on a PE-transposed [128 tokens, 8, 14]
    layout so the elementwise chain uses all 128 lanes.
"""

import numpy as np

B, C, T = 2, 512, 4096
D = 14
NCORES = 8
TCORE = (B * T) // NCORES  # 1024
CHUNK = 512
NCHUNK = TCORE // CHUNK
NTILE = TCORE // 128
INV_TEMP = 100.0
ENTROPY_W = 0.1
COMMIT_W = 0.25
DIVERSITY_GAMMA = 1.0
EPS = 1e-20

_CACHE = {}


def _build_module():
    import concourse.bacc as bacc
    import concourse.bass as bass
    import concourse.mybir as mybir
    import concourse.tile as tile

    f32 = mybir.dt.float32
    bf16 = mybir.dt.bfloat16
    Act = mybir.ActivationFunctionType
    Alu = mybir.AluOpType
    X = mybir.AxisListType.X
    XY = mybir.AxisListType.XY

    nc = bacc.Bacc("TRN2", target_bir_lowering=False, debug=False,
                   num_devices=NCORES)

    ze_d = nc.dram_tensor("z_part", (C, TCORE), f32, kind="ExternalInput")
    win_d = nc.dram_tensor("w_in_c", (128, 4 * D), f32, kind="ExternalInput")
    wout_d = nc.dram_tensor("w_outT", (D + 1, C), bf16, kind="ExternalInput")
    cst_d = nc.dram_tensor("consts", (D, 273), f32, kind="ExternalInput")
    ones_d = nc.dram_tensor("ones_row", (1, TCORE), bf16, kind="ExternalInput")

    zq_d = nc.dram_tensor("zq_part", (C, TCORE), f32, kind="ExternalOutput")
    avg_d = nc.dram_tensor("avg_part", (128, 128), f32, kind="ExternalOutput")
    stats_d = nc.dram_tensor("stats", (128, 2), f32, kind="ExternalOutput")
    idx_d = nc.dram_tensor("idx", (1, TCORE), f32, kind="ExternalOutput")

    ze_r = ze_d.ap().rearrange("(c p) (j t) -> p c j t", p=128, t=CHUNK)
    zq_r = zq_d.ap().rearrange("(c p) (j t) -> p c j t", p=128, t=CHUNK)

    with tile.TileContext(nc) as tc:
        with (
            tc.tile_pool(name="const", bufs=1) as cpool,
            tc.tile_pool(name="data", bufs=1) as dpool,
            tc.tile_pool(name="chunkio", bufs=2) as iopool,
            tc.tile_pool(name="work", bufs=2) as wpool,
            tc.tile_pool(name="ab", bufs=3) as abpool,
            tc.tile_pool(name="ps", bufs=2, space=bass.MemorySpace.PSUM) as pspool,
            tc.tile_pool(name="ps1", bufs=2, space=bass.MemorySpace.PSUM) as ps1pool,
            tc.tile_pool(name="psy", bufs=1, space=bass.MemorySpace.PSUM) as psypool,
            tc.tile_pool(name="psavg", bufs=1, space=bass.MemorySpace.PSUM) as avgpool,
        ):
            # ---- PE warmup: dummy matmuls on a memset tile while z_e lands
            warm_t = cpool.tile([128, 256], f32)
            nc.gpsimd.memset(warm_t[:], 0.125)
            for wi in range(8):
                wp = pspool.tile([128, 256], f32, tag="Lpsum")
                nc.tensor.matmul(wp[:], warm_t[:, 0:128], warm_t[:])

            # ---- input DMAs: z_e chunks on the SP ring, consts on SWDGE
            ze_ts = []
            for j in range(NCHUNK):
                ze_t = iopool.tile([128, 4, CHUNK], f32, tag="ze")
                for c in range(4):
                    nc.sync.dma_start(ze_t[:, c, :], ze_r[:, c, j])
                ze_ts.append(ze_t)

            win_t = cpool.tile([128, 4 * D], f32)
            nc.gpsimd.dma_start(win_t[:], win_d.ap())
            wout_t = cpool.tile([D + 1, C], bf16)
            nc.gpsimd.dma_start(wout_t[:], wout_d.ap())
            cst_t = cpool.tile([D, 273], f32)
            nc.gpsimd.dma_start(cst_t[:], cst_d.ap())

            ones_bf = cpool.tile([D, 1], bf16)
            nc.gpsimd.memset(ones_bf[:], 1.0)

            y_t = dpool.tile([D, TCORE], f32)
            q_t = dpool.tile([D + 1, TCORE], bf16)
            nc.gpsimd.dma_start(q_t[D:D + 1, :], ones_d.ap())
            idx_t = dpool.tile([1, TCORE], f32)
            avg_ps = avgpool.tile([128, 128], f32)

            for j in range(NCHUNK):
                cs = slice(CHUNK * j, CHUNK * (j + 1))
                ze_t = ze_ts[j]

                # ---- project in: y[d, t] = sum_c W_in[d, c] z_e[c, t] ----
                yp = psypool.tile([D, CHUNK], f32, tag="ypsum")
                for c in range(4):
                    nc.tensor.matmul(yp[:], win_t[:, D * c:D * (c + 1)],
                                     ze_t[:, c, :],
                                     start=(c == 0), stop=(c == 3))
                nc.vector.tensor_scalar_add(y_t[:, cs], yp[:],
                                            cst_t[:, 258:259])

                # ---- quantize; weighted bits for the hard index ----
                bits_t = wpool.tile([D, CHUNK], f32, tag="bits")
                nc.vector.tensor_scalar(bits_t[:], y_t[:, cs], 0.0, None,
                                        op0=Alu.is_gt)
                nc.vector.tensor_scalar(q_t[0:D, cs], bits_t[:], 2.0, -1.0,
                                        op0=Alu.mult, op1=Alu.add)
                wb_t = wpool.tile([D, CHUNK], bf16, tag="wb")
                nc.vector.tensor_scalar_mul(wb_t[:], bits_t[:],
                                            cst_t[:, 256:257])

                # ---- project out (bf16; b_out folded in as 15th row) ----
                zq_t = iopool.tile([128, 4, CHUNK], f32, tag="zq")
                for c in range(4):
                    zp = pspool.tile([128, CHUNK], f32, tag="zqpsum")
                    nc.tensor.matmul(zp[:], wout_t[:, 128 * c:128 * (c + 1)],
                                     q_t[:, cs])
                    if c % 2 == 0:
                        nc.scalar.copy(zq_t[:, c, :], zp[:])
                    else:
                        nc.vector.tensor_copy(zq_t[:, c, :], zp[:])
                nc.scalar.dma_start(zq_r[:, :, j], zq_t[:])

                # ---- hard index: sum_d bit_d * 2^(13-d) ----
                ip = ps1pool.tile([1, CHUNK], f32, tag="idxpsum")
                nc.tensor.matmul(ip[:], ones_bf[:], wb_t[:])
                nc.vector.tensor_copy(idx_t[:, cs], ip[:])

                # ---- A/B half-softmaxes; avg_prob outer-product partial ----
                for tt in range(CHUNK // 128):
                    gt = j * (CHUNK // 128) + tt
                    Lp = pspool.tile([128, 2, 128], f32, tag="Lpsum")
                    nc.tensor.matmul(
                        Lp[:].rearrange("p a b -> p (a b)"),
                        y_t[:, 128 * gt:128 * (gt + 1)],
                        cst_t[:, 0:256],
                    )
                    m2 = abpool.tile([128, 2], f32, tag="m2")
                    nc.vector.tensor_reduce(m2[:], Lp[:], axis=X,
                                            op=Alu.max, negate=True)
                    E_t = abpool.tile([128, 2, 128], bf16, tag="E")
                    s2 = abpool.tile([128, 2], f32, tag="s2")
                    for h in range(2):
                        nc.scalar.activation(E_t[:, h, :], Lp[:, h, :], Act.Exp,
                                             bias=m2[:, h:h + 1],
                                             accum_out=s2[:, h:h + 1])
                    sprod = abpool.tile([128, 1], f32, tag="sprod")
                    nc.vector.tensor_tensor(sprod[:], s2[:, 0:1], s2[:, 1:2],
                                            op=Alu.mult)
                    rr = abpool.tile([128, 1], f32, tag="rr")
                    nc.vector.reciprocal(rr[:], sprod[:])
                    Ap_t = abpool.tile([128, 128], bf16, tag="Ap")
                    nc.vector.tensor_scalar_mul(Ap_t[:], E_t[:, 0, :], rr[:])
                    nc.tensor.matmul(avg_ps[:], Ap_t[:], E_t[:, 1, :],
                                     start=(gt == 0), stop=(gt == NTILE - 1))

            # ---- transpose y to [128 tokens, NTILE, 14] for cheap stats ----
            yT_t = dpool.tile([128, NTILE, D], f32)
            for gt in range(NTILE):
                tp = ps1pool.tile([128, D], f32, tag="idxpsum")
                nc.tensor.transpose(tp[:], y_t[:, 128 * gt:128 * (gt + 1)],
                                    cst_t[:, 259:273])
                if gt % 2 == 0:
                    nc.scalar.copy(yT_t[:, gt, :], tp[:])
                else:
                    nc.vector.tensor_copy(yT_t[:, gt, :], tp[:])

            # ---- Bernoulli entropy + commitment partial sums ----
            # p = sigmoid(400 y);  H_b = -ln(max(p,1-p)) + 400|y| min(p,1-p)
            # (log always of the large branch -> stays in the ACT Ln LUT's
            #  accurate range; sigmoid saturation gives exactly H=0)
            sh = [128, NTILE, D]
            p_t = wpool.tile(sh, f32, tag="p")
            nc.scalar.activation(p_t[:], yT_t[:], Act.Sigmoid, scale=400.0)
            q1_t = wpool.tile(sh, f32, tag="q1")
            nc.vector.tensor_scalar(q1_t[:], p_t[:], -1.0, 1.0,
                                    op0=Alu.mult, op1=Alu.add)
            pm_t = wpool.tile(sh, f32, tag="pm")
            nc.vector.tensor_tensor(pm_t[:], p_t[:], q1_t[:], op=Alu.max)
            pn_t = wpool.tile(sh, f32, tag="pn")
            nc.vector.tensor_tensor(pn_t[:], p_t[:], q1_t[:], op=Alu.min)
            lp_t = wpool.tile(sh, f32, tag="lp")
            nc.scalar.activation(lp_t[:], pm_t[:], Act.Ln)
            n_t = wpool.tile(sh, f32, tag="n")
            nc.vector.tensor_scalar_mul(n_t[:], yT_t[:], -1.0)
            a_t = wpool.tile(sh, f32, tag="a")
            nc.vector.tensor_tensor(a_t[:], yT_t[:], n_t[:], op=Alu.max)
            t_t = wpool.tile(sh, f32, tag="t")
            nc.vector.tensor_tensor(t_t[:], a_t[:], pn_t[:], op=Alu.mult)
            scr_t = wpool.tile(sh, f32, tag="scr")
            nc.vector.scalar_tensor_tensor(scr_t[:], t_t[:], 400.0, lp_t[:],
                                           op0=Alu.mult, op1=Alu.subtract)
            stats_t = dpool.tile([128, 2], f32)
            nc.vector.tensor_reduce(stats_t[:, 0:1], scr_t[:], axis=XY,
                                    op=Alu.add)
            c1_t = wpool.tile(sh, f32, tag="c1")
            nc.vector.tensor_scalar_add(c1_t[:], a_t[:], -1.0)
            sq_t = wpool.tile(sh, f32, tag="sq")
            nc.scalar.activation(sq_t[:], c1_t[:], Act.Square,
                                 accum_out=stats_t[:, 1:2])

            avg_t = wpool.tile([128, 128], f32)
            nc.vector.tensor_copy(avg_t[:], avg_ps[:])
            nc.scalar.dma_start(avg_d.ap(), avg_t[:])
            nc.scalar.dma_start(stats_d.ap(), stats_t[:])
            nc.scalar.dma_start(idx_d.ap(), idx_t[:])

    nc.compile()
    return nc


def _host_consts(b_in):
    # consts layout (14, 273):
    #   [:, 0:256]  block-diagonal scaled half-codebooks:
    #       rows 0:7  cols   0:128 = 200 * C7T ; rows 7:14 cols 128:256 = 200 * C7T
    #   [:, 256] 2^(13-d)   [:, 257] unused   [:, 258] b_in
    #   [:, 259:273] 14x14 identity (for PE transpose)
    c7t = ((((np.arange(128)[None, :] >> np.arange(6, -1, -1)[:, None]) & 1)
            * 2.0 - 1.0)).astype(np.float32)  # (7, 128)
    consts = np.zeros((D, 273), np.float32)
    consts[0:7, 0:128] = 2.0 * INV_TEMP * c7t
    consts[7:14, 128:256] = 2.0 * INV_TEMP * c7t
    consts[:, 256] = (1 << np.arange(D - 1, -1, -1)).astype(np.float32)
    consts[:, 258] = b_in
    consts[:, 259:273] = np.eye(D, dtype=np.float32)
    return consts


def _host_inputs(z_e, W_in, b_in, W_out, b_out):
    import ml_dtypes
    w_in_c = np.ascontiguousarray(
        W_in.T.reshape(4, 128, D).transpose(1, 0, 2).reshape(128, 4 * D))
    w_outT = np.empty((D + 1, C), np.float32)
    w_outT[0:D] = W_out.T
    w_outT[D] = b_out
    w_outT = w_outT.astype(ml_dtypes.bfloat16)
    consts = _host_consts(b_in)
    ones = np.ones((1, TCORE), ml_dtypes.bfloat16)
    in_maps = []
    for k in range(NCORES):
        b, s = divmod(k, NCORES // B)
        zp = np.ascontiguousarray(z_e[b, :, s * TCORE:(s + 1) * TCORE])
        in_maps.append({"z_part": zp, "w_in_c": w_in_c, "w_outT": w_outT,
                        "consts": consts, "ones_row": ones})
    return in_maps


def kernel(z_e, W_in, b_in, W_out, b_out):
    from concourse import bass_utils

    z_e = np.ascontiguousarray(np.asarray(z_e, np.float32))
    W_in = np.asarray(W_in, np.float32)
    b_in = np.asarray(b_in, np.float32)
    W_out = np.asarray(W_out, np.float32)
    b_out = np.asarray(b_out, np.float32)

    if "nc" not in _CACHE:
        _CACHE["nc"] = _build_module()
    nc = _CACHE["nc"]

    in_maps = _host_inputs(z_e, W_in, b_in, W_out, b_out)
    res = bass_utils.run_bass_kernel_spmd(nc, in_maps, core_ids=list(range(NCORES)))
    results = res.results

    z_q = np.empty((B, C, T), np.float32)
    avg_sum = np.zeros((128, 128), np.float64)
    ent_sum = 0.0
    commit_sum = 0.0
    idx_all = []
    for k in range(NCORES):
        b, s = divmod(k, NCORES // B)
        r = results[k]
        z_q[b, :, s * TCORE:(s + 1) * TCORE] = r["zq_part"]
        avg_sum += r["avg_part"].astype(np.float64)
        ent_sum += float(r["stats"][:, 0].sum(dtype=np.float64))
        commit_sum += float(r["stats"][:, 1].sum(dtype=np.float64))
        idx_all.append(r["idx"].ravel())

    n = B * T
    avg_prob = avg_sum / n
    cb_ent = float(-np.sum(avg_prob * np.log(np.clip(avg_prob, EPS, None))))
    ps_ent = ent_sum / n
    commit = commit_sum / (n * D)
    aux = (ps_ent - DIVERSITY_GAMMA * cb_ent) * ENTROPY_W + commit * COMMIT_W

    idx = np.concatenate(idx_all).astype(np.int64)
    usage = len(np.unique(idx)) / 16384.0

    return (z_q, np.float32(aux), np.float32(usage))


# revision 28
# speedup vs baseline: 1.1615x; 1.0074x over previous
"""Trainium2 Bass kernel for LucidrainsLFQ (lookup-free quantization).

Reference computation (per token, C=512 channels, D=14 codebook bits):
  y      = W_in @ z_e + b_in                      (project in, 14 dims)
  quant  = sign(y)                                (+-1)
  z_q    = W_out @ quant + b_out                  (project out)
  The softmax over the 2^14 implicit codebook factorizes: the codebook is
  all sign patterns of {-1,+1}^14, so softmax(200 * y . c_j) over j is a
  product of 14 independent Bernoullis.  Split 14 bits = 7 + 7:
     prob[jA, jB] = A[jA] * B[jB]   with A, B 128-way softmaxes
  avg_prob = mean_n A_n outer B_n = (A^T B)/N  -> one 128x128 matmul.
  per-sample entropy = sum_d H_b(sigmoid(400 y_d))  (Bernoulli entropies)
  commit = mean((|y| - 1)^2)
  usage  = distinct hard codes / 16384

Sharding: data-parallel over the 8192 tokens; each of 8 cores handles one
(batch b, 1024-token slice).  Tiny weights/codebook are replicated.  Each
core returns its z_q slice plus partial sums (avg-prob outer product,
entropy/commit sums, hard indices); the host combines the partials.

Performance structure:
  * two 512-token chunks pipelined; z_e arrives in 256 KB per-C-chunk DMAs
    so the first in-projection matmul starts as early as possible; consts
    ride the GpSimd SWDGE ring so they don't serialize behind z_e on SP.
  * dummy matmuls warm the PE clock (HAM) during the initial DMA window.
  * out-projection runs in bf16 (quant is exactly representable; W_out
    rounding contributes ~4e-4 relative on z_q); b_out is folded in as a
    15th contraction row whose activation is constant 1.
  * entropy/commit run post-loop on a PE-transposed [128 tokens, 8, 14]
    layout so the elementwise chain uses all 128 lanes.
"""

import numpy as np

B, C, T = 2, 512, 4096
D = 14
NCORES = 8
TCORE = (B * T) // NCORES  # 1024
CHUNK = 512
NCHUNK = TCORE // CHUNK
NTILE = TCORE // 128
INV_TEMP = 100.0
ENTROPY_W = 0.1
COMMIT_W = 0.25
DIVERSITY_GAMMA = 1.0
EPS = 1e-20

_CACHE = {}


def _build_module():
    import concourse.bacc as bacc
    import concourse.bass as bass
    import concourse.mybir as mybir
    import concourse.tile as tile

    f32 = mybir.dt.float32
    bf16 = mybir.dt.bfloat16
    Act = mybir.ActivationFunctionType
    Alu = mybir.AluOpType
    X = mybir.AxisListType.X
    XY = mybir.AxisListType.XY

    nc = bacc.Bacc("TRN2", target_bir_lowering=False, debug=False,
                   num_devices=NCORES)

    ze_d = nc.dram_tensor("z_part", (C, TCORE), f32, kind="ExternalInput")
    win_d = nc.dram_tensor("w_in_c", (128, 4 * D), f32, kind="ExternalInput")
    wout_d = nc.dram_tensor("w_outT", (D + 1, C), bf16, kind="ExternalInput")
    cst_d = nc.dram_tensor("consts", (D, 273), f32, kind="ExternalInput")
    cbk_d = nc.dram_tensor("cbk_bf", (D, 256), bf16, kind="ExternalInput")
    ones_d = nc.dram_tensor("ones_row", (1, TCORE), bf16, kind="ExternalInput")

    zq_d = nc.dram_tensor("zq_part", (C, TCORE), f32, kind="ExternalOutput")
    avg_d = nc.dram_tensor("avg_part", (128, 128), f32, kind="ExternalOutput")
    stats_d = nc.dram_tensor("stats", (128, 2 * NCHUNK), f32,
                             kind="ExternalOutput")
    idx_d = nc.dram_tensor("idx", (1, TCORE), f32, kind="ExternalOutput")

    ze_r = ze_d.ap().rearrange("(c p) (j t) -> p c j t", p=128, t=CHUNK)
    zq_r = zq_d.ap().rearrange("(c p) (j t) -> p c j t", p=128, t=CHUNK)

    with tile.TileContext(nc) as tc:
        with (
            tc.tile_pool(name="const", bufs=1) as cpool,
            tc.tile_pool(name="data", bufs=1) as dpool,
            tc.tile_pool(name="chunkio", bufs=2) as iopool,
            tc.tile_pool(name="work", bufs=2) as wpool,
            tc.tile_pool(name="ab", bufs=3) as abpool,
            tc.tile_pool(name="ps", bufs=2, space=bass.MemorySpace.PSUM) as pspool,
            tc.tile_pool(name="ps1", bufs=2, space=bass.MemorySpace.PSUM) as ps1pool,
            tc.tile_pool(name="psy", bufs=1, space=bass.MemorySpace.PSUM) as psypool,
            tc.tile_pool(name="psavg", bufs=1, space=bass.MemorySpace.PSUM) as avgpool,
        ):
            # ---- PE warmup: dummy matmuls on a memset tile while z_e lands
            warm_t = cpool.tile([128, 256], f32)
            nc.gpsimd.memset(warm_t[:], 0.125)
            for wi in range(4):
                wp = pspool.tile([128, 256], f32, tag="Lpsum")
                nc.tensor.matmul(wp[:], warm_t[:, 0:128], warm_t[:])

            # ---- input DMAs: z_e chunks on the SP ring, consts on SWDGE
            ze_ts = []
            for j in range(NCHUNK):
                ze_t = iopool.tile([128, 4, CHUNK], f32, tag="ze")
                for c in range(4):
                    nc.sync.dma_start(ze_t[:, c, :], ze_r[:, c, j])
                ze_ts.append(ze_t)

            win_t = cpool.tile([128, 4 * D], f32)
            nc.gpsimd.dma_start(win_t[:], win_d.ap())
            wout_t = cpool.tile([D + 1, C], bf16)
            nc.gpsimd.dma_start(wout_t[:], wout_d.ap())
            cst_t = cpool.tile([D, 273], f32)
            nc.gpsimd.dma_start(cst_t[:], cst_d.ap())
            cbk_t = cpool.tile([D, 256], bf16)
            nc.gpsimd.dma_start(cbk_t[:], cbk_d.ap())

            ones_bf = cpool.tile([D, 1], bf16)
            nc.gpsimd.memset(ones_bf[:], 1.0)

            y_t = dpool.tile([D, TCORE], f32)
            yhi_t = dpool.tile([D, TCORE], bf16)
            ylo_t = dpool.tile([D, TCORE], bf16)
            yT_t = dpool.tile([128, NTILE, D], f32)
            stats_t = dpool.tile([128, 2 * NCHUNK], f32)
            q_t = dpool.tile([D + 1, TCORE], bf16)
            nc.gpsimd.dma_start(q_t[D:D + 1, :], ones_d.ap())
            idx_t = dpool.tile([1, TCORE], f32)
            avg_ps = avgpool.tile([128, 128], f32)

            for j in range(NCHUNK):
                cs = slice(CHUNK * j, CHUNK * (j + 1))
                ze_t = ze_ts[j]

                # ---- project in: y[d, t] = sum_c W_in[d, c] z_e[c, t] ----
                yp = psypool.tile([D, CHUNK], f32, tag="ypsum")
                for c in range(4):
                    nc.tensor.matmul(yp[:], win_t[:, D * c:D * (c + 1)],
                                     ze_t[:, c, :],
                                     start=(c == 0), stop=(c == 3))
                nc.vector.tensor_scalar_add(y_t[:, cs], yp[:],
                                            cst_t[:, 258:259])
                # bf16 hi/lo split of y for single-pass logit matmuls
                nc.scalar.copy(yhi_t[:, cs], y_t[:, cs])
                nc.vector.tensor_tensor(ylo_t[:, cs], y_t[:, cs],
                                        yhi_t[:, cs], op=Alu.subtract)

                # ---- quantize; weighted bits for the hard index ----
                bits_t = wpool.tile([D, CHUNK], f32, tag="bits")
                nc.vector.tensor_scalar(bits_t[:], y_t[:, cs], 0.0, None,
                                        op0=Alu.is_gt)
                nc.vector.tensor_scalar(q_t[0:D, cs], bits_t[:], 2.0, -1.0,
                                        op0=Alu.mult, op1=Alu.add)
                wb_t = wpool.tile([D, CHUNK], bf16, tag="wb")
                nc.vector.tensor_scalar_mul(wb_t[:], bits_t[:],
                                            cst_t[:, 256:257])

                # ---- project out (bf16; b_out folded in as 15th row) ----
                zq_t = iopool.tile([128, 4, CHUNK], f32, tag="zq")
                for c in range(4):
                    zp = pspool.tile([128, CHUNK], f32, tag="zqpsum")
                    nc.tensor.matmul(zp[:], wout_t[:, 128 * c:128 * (c + 1)],
                                     q_t[:, cs])
                    if c % 2 == 0:
                        nc.scalar.copy(zq_t[:, c, :], zp[:])
                    else:
                        nc.vector.tensor_copy(zq_t[:, c, :], zp[:])
                nc.scalar.dma_start(zq_r[:, :, j], zq_t[:])

                # ---- hard index: sum_d bit_d * 2^(13-d) ----
                ip = ps1pool.tile([1, CHUNK], f32, tag="idxpsum")
                nc.tensor.matmul(ip[:], ones_bf[:], wb_t[:])
                nc.vector.tensor_copy(idx_t[:, cs], ip[:])

                # ---- A/B half-softmaxes; avg_prob outer-product partial ----
                for tt in range(CHUNK // 128):
                    gt = j * (CHUNK // 128) + tt
                    ts = slice(128 * gt, 128 * (gt + 1))
                    Lp = pspool.tile([128, 2, 128], f32, tag="Lpsum")
                    Lpf = Lp[:].rearrange("p a b -> p (a b)")
                    nc.tensor.matmul(Lpf, yhi_t[:, ts], cbk_t[:],
                                     start=True, stop=False)
                    nc.tensor.matmul(Lpf, ylo_t[:, ts], cbk_t[:],
                                     start=False, stop=True)
                    m2 = abpool.tile([128, 2], f32, tag="m2")
                    nc.vector.tensor_reduce(m2[:], Lp[:], axis=X,
                                            op=Alu.max, negate=True)
                    E_t = abpool.tile([128, 2, 128], bf16, tag="E")
                    s2 = abpool.tile([128, 2], f32, tag="s2")
                    for h in range(2):
                        nc.scalar.activation(E_t[:, h, :], Lp[:, h, :], Act.Exp,
                                             bias=m2[:, h:h + 1],
                                             accum_out=s2[:, h:h + 1])
                    sprod = abpool.tile([128, 1], f32, tag="sprod")
                    nc.vector.tensor_tensor(sprod[:], s2[:, 0:1], s2[:, 1:2],
                                            op=Alu.mult)
                    rr = abpool.tile([128, 1], f32, tag="rr")
                    nc.vector.reciprocal(rr[:], sprod[:])
                    Ap_t = abpool.tile([128, 128], bf16, tag="Ap")
                    nc.vector.tensor_scalar_mul(Ap_t[:], E_t[:, 0, :], rr[:])
                    nc.tensor.matmul(avg_ps[:], Ap_t[:], E_t[:, 1, :],
                                     start=(gt == 0), stop=(gt == NTILE - 1))

                # ---- transpose this chunk's y to [128, 4, 14] ----
                for tt in range(CHUNK // 128):
                    gt = j * (CHUNK // 128) + tt
                    tp = ps1pool.tile([128, D], f32, tag="idxpsum")
                    nc.tensor.transpose(tp[:], y_t[:, 128 * gt:128 * (gt + 1)],
                                        cst_t[:, 259:273])
                    if gt % 2 == 0:
                        nc.scalar.copy(yT_t[:, gt, :], tp[:])
                    else:
                        nc.vector.tensor_copy(yT_t[:, gt, :], tp[:])

                # ---- Bernoulli entropy + commitment partial sums ----
                # p = sigmoid(400 y); H_b = -ln(max(p,1-p)) + 400|y| min(p,1-p)
                # (log always of the large branch -> stays in the ACT Ln
                #  LUT's accurate range; sigmoid saturation gives H=0 exact)
                yTj = yT_t[:, 4 * j:4 * (j + 1), :]
                sh = [128, CHUNK // 128, D]
                p_t = wpool.tile(sh, f32, tag="p")
                nc.scalar.activation(p_t[:], yTj, Act.Sigmoid, scale=400.0)
                q1_t = wpool.tile(sh, f32, tag="q1")
                nc.vector.tensor_scalar(q1_t[:], p_t[:], -1.0, 1.0,
                                        op0=Alu.mult, op1=Alu.add)
                pm_t = wpool.tile(sh, f32, tag="pm")
                nc.vector.tensor_tensor(pm_t[:], p_t[:], q1_t[:], op=Alu.max)
                pn_t = wpool.tile(sh, f32, tag="pn")
                nc.vector.tensor_tensor(pn_t[:], p_t[:], q1_t[:], op=Alu.min)
                lp_t = wpool.tile(sh, f32, tag="lp")
                nc.scalar.activation(lp_t[:], pm_t[:], Act.Ln)
                n_t = wpool.tile(sh, f32, tag="n")
                nc.vector.tensor_scalar_mul(n_t[:], yTj, -1.0)
                a_t = wpool.tile(sh, f32, tag="a")
                nc.vector.tensor_tensor(a_t[:], yTj, n_t[:], op=Alu.max)
                t_t = wpool.tile(sh, f32, tag="t")
                nc.vector.tensor_tensor(t_t[:], a_t[:], pn_t[:], op=Alu.mult)
                scr_t = wpool.tile(sh, f32, tag="scr")
                nc.vector.scalar_tensor_tensor(scr_t[:], t_t[:], 400.0,
                                               lp_t[:], op0=Alu.mult,
                                               op1=Alu.subtract)
                nc.vector.tensor_reduce(stats_t[:, j:j + 1], scr_t[:],
                                        axis=XY, op=Alu.add)
                c1_t = wpool.tile(sh, f32, tag="c1")
                nc.vector.tensor_scalar_add(c1_t[:], a_t[:], -1.0)
                sq_t = wpool.tile(sh, f32, tag="sq")
                nc.scalar.activation(
                    sq_t[:], c1_t[:], Act.Square,
                    accum_out=stats_t[:, NCHUNK + j:NCHUNK + j + 1])

            avg_t = wpool.tile([128, 128], f32)
            nc.vector.tensor_copy(avg_t[:], avg_ps[:])
            nc.scalar.dma_start(avg_d.ap(), avg_t[:])
            nc.scalar.dma_start(stats_d.ap(), stats_t[:])
            nc.scalar.dma_start(idx_d.ap(), idx_t[:])

    nc.compile()
    return nc


def _host_consts(b_in):
    # consts layout (14, 273):
    #   [:, 0:256]  block-diagonal scaled half-codebooks:
    #       rows 0:7  cols   0:128 = 200 * C7T ; rows 7:14 cols 128:256 = 200 * C7T
    #   [:, 256] 2^(13-d)   [:, 257] unused   [:, 258] b_in
    #   [:, 259:273] 14x14 identity (for PE transpose)
    c7t = ((((np.arange(128)[None, :] >> np.arange(6, -1, -1)[:, None]) & 1)
            * 2.0 - 1.0)).astype(np.float32)  # (7, 128)
    consts = np.zeros((D, 273), np.float32)
    consts[0:7, 0:128] = 2.0 * INV_TEMP * c7t
    consts[7:14, 128:256] = 2.0 * INV_TEMP * c7t
    consts[:, 256] = (1 << np.arange(D - 1, -1, -1)).astype(np.float32)
    consts[:, 258] = b_in
    consts[:, 259:273] = np.eye(D, dtype=np.float32)
    return consts


def _host_inputs(z_e, W_in, b_in, W_out, b_out):
    import ml_dtypes
    w_in_c = np.ascontiguousarray(
        W_in.T.reshape(4, 128, D).transpose(1, 0, 2).reshape(128, 4 * D))
    w_outT = np.empty((D + 1, C), np.float32)
    w_outT[0:D] = W_out.T
    w_outT[D] = b_out
    w_outT = w_outT.astype(ml_dtypes.bfloat16)
    consts = _host_consts(b_in)
    cbk_bf = consts[:, 0:256].astype(ml_dtypes.bfloat16)
    ones = np.ones((1, TCORE), ml_dtypes.bfloat16)
    in_maps = []
    for k in range(NCORES):
        b, s = divmod(k, NCORES // B)
        zp = np.ascontiguousarray(z_e[b, :, s * TCORE:(s + 1) * TCORE])
        in_maps.append({"z_part": zp, "w_in_c": w_in_c, "w_outT": w_outT,
                        "consts": consts, "cbk_bf": cbk_bf, "ones_row": ones})
    return in_maps


def kernel(z_e, W_in, b_in, W_out, b_out):
    from concourse import bass_utils

    z_e = np.ascontiguousarray(np.asarray(z_e, np.float32))
    W_in = np.asarray(W_in, np.float32)
    b_in = np.asarray(b_in, np.float32)
    W_out = np.asarray(W_out, np.float32)
    b_out = np.asarray(b_out, np.float32)

    if "nc" not in _CACHE:
        _CACHE["nc"] = _build_module()
    nc = _CACHE["nc"]

    in_maps = _host_inputs(z_e, W_in, b_in, W_out, b_out)
    res = bass_utils.run_bass_kernel_spmd(nc, in_maps, core_ids=list(range(NCORES)))
    results = res.results

    z_q = np.empty((B, C, T), np.float32)
    avg_sum = np.zeros((128, 128), np.float64)
    ent_sum = 0.0
    commit_sum = 0.0
    idx_all = []
    for k in range(NCORES):
        b, s = divmod(k, NCORES // B)
        r = results[k]
        z_q[b, :, s * TCORE:(s + 1) * TCORE] = r["zq_part"]
        avg_sum += r["avg_part"].astype(np.float64)
        ent_sum += float(r["stats"][:, 0:NCHUNK].sum(dtype=np.float64))
        commit_sum += float(r["stats"][:, NCHUNK:].sum(dtype=np.float64))
        idx_all.append(r["idx"].ravel())

    n = B * T
    avg_prob = avg_sum / n
    cb_ent = float(-np.sum(avg_prob * np.log(np.clip(avg_prob, EPS, None))))
    ps_ent = ent_sum / n
    commit = commit_sum / (n * D)
    aux = (ps_ent - DIVERSITY_GAMMA * cb_ent) * ENTROPY_W + commit * COMMIT_W

    idx = np.concatenate(idx_all).astype(np.int64)
    usage = len(np.unique(idx)) / 16384.0

    return (z_q, np.float32(aux), np.float32(usage))
